# revision 28
# baseline (speedup 1.0000x reference)
"""GPT (4-layer, C=512, H=8, T=1024, B=2, V=50257, LoRA r=8) on 8 trn2 cores.

Sharding: 2 groups of 4 cores (one per batch element); sequence-parallel
within a group (each core owns 256 tokens) with a per-layer KV AllGather;
vocab-sharded head matmul after a final all-core AllGather of x.
SPMD-uniform program: rank differences live in host-side data (causal masks,
padded head shards).

Optimizations over the first working version: bf16 head output (host
casts back to fp32), batched head output DMAs (4 token tiles per
transfer), multiplicative bf16 causal masks applied post-exp on DVE
instead of additive fp32 PSUM RMW, layernorm variance via a single DVE
scalar_tensor_tensor with accumulate (no ACT Square + table swap),
K/V-first qkv chunk ordering so the KV AllGather overlaps the Q
projection matmuls, single-tile kt_all/v_aug/xT_all with batched
collective-unpack DMAs (8 instead of 24 per layer), and double-buffered
weight pools for cross-layer DMA prefetch.
"""
import math
import numpy as np
import ml_dtypes

import concourse.bass as bass
import concourse.bacc as bacc
import concourse.tile as tile
import concourse.mybir as mybir
from concourse import bass_utils

BF16 = mybir.dt.bfloat16
F32 = mybir.dt.float32
AF = mybir.ActivationFunctionType

L, H, C, V, B, T = 4, 8, 512, 50257, 2, 1024
R = 8
NCORES = 8
TO = 256            # tokens owned per core
NTT = TO // 128     # 2 token tiles per core
NF = C // 128       # 4 feature tiles
HD = C // H         # 64 head dim
VC = 6283           # padded vocab shard (8*6283 = 50264 >= 50257)
NVCH = 13           # vocab chunks of 512 (last = 139)

_CACHE = {}


def build_nc(debug=False):
    nc = bacc.Bacc("TRN2", target_bir_lowering=False, debug=False,
                   num_devices=NCORES)
    d = {}
    def inp(name, shape, dt):
        d[name] = nc.dram_tensor(name, shape, dt, kind="ExternalInput").ap()
    inp("x0", [NTT, 128, C], F32)
    inp("masks", [8, 128, TO], BF16)    # multiplicative {0,1}
    inp("ident", [128, 128], BF16)
    inp("aw", [L, C, 3 * C], BF16)     # attn_w.T, q-cols pre-scaled
    inp("ala", [L, C, R], BF16)
    inp("alb", [L, R, 3 * C], BF16)    # *4.0, q-cols pre-scaled
    inp("pw", [L, C, C], BF16)
    inp("pla", [L, C, R], BF16)
    inp("plb", [L, R, C], BF16)        # *4.0
    inp("fw", [L, C, 4 * C], BF16)
    inp("mw", [L, 4 * C, C], BF16)
    inp("hw", [C, VC], BF16)           # head shard (rank-dep, zero-padded)
    y_d = nc.dram_tensor("y", [8 * TO, VC], BF16, kind="ExternalOutput").ap()
    if debug:
        xdbg = nc.dram_tensor("xdbg", [L, NTT, 128, C], F32,
                              kind="ExternalOutput").ap()

    with tile.TileContext(nc) as tc:
        with (
            tc.tile_pool(name="persist", bufs=1) as pp,
            tc.tile_pool(name="wts", bufs=1) as wp,
            tc.tile_pool(name="acts", bufs=1) as ap_,
            tc.tile_pool(name="acts3", bufs=3) as ap3,
            tc.tile_pool(name="stats", bufs=3) as sp,
            tc.tile_pool(name="dram", bufs=2, space="DRAM") as dp,
            tc.tile_pool(name="psu", bufs=8, space="PSUM") as psu,
        ):
            ident = pp.tile([128, 128], BF16)
            nc.sync.dma_start(ident[:], d["ident"][:])
            zt = pp.tile([128, 1], F32)
            nc.vector.memset(zt[:], 0.0)
            eps = pp.tile([128, 1], F32)
            nc.vector.memset(eps[:], 1e-5)
            maskT = pp.tile([128, 8, TO], BF16)
            nc.sync.dma_start(maskT[:], d["masks"].rearrange("k p q -> p k q"))

            x = [pp.tile([128, C], F32, name=f"x{tt}", tag=f"x{tt}") for tt in range(NTT)]

            kt_all = pp.tile([128, NF, T], BF16, name="kt_all", tag="kt_all")
            v_aug = pp.tile([128, T // 128, H, HD + 1], BF16, name="v_aug",
                            tag="v_aug")
            nc.vector.memset(v_aug[:, :, :, HD:HD + 1], 1.0)

            def layernorm(src_tiles):
                """Return bf16 normalized tiles (gamma folded on host, beta==0)."""
                out = []
                for tt in range(NTT):
                    nm = sp.tile([128, 1], F32, name="nm", tag="nm")
                    nc.vector.reduce_sum(nm[:], src_tiles[tt][:],
                                         axis=mybir.AxisListType.X, negate=True)
                    nms = sp.tile([128, 1], F32, name="nms", tag="nms")
                    nc.vector.tensor_scalar_mul(nms[:], nm[:], 1.0 / C)
                    xc = ap_.tile([128, C], F32, name="xc", tag="xc")
                    nc.vector.tensor_scalar_add(xc[:], src_tiles[tt][:], nms[:])
                    sq = ap_.tile([128, C], BF16, name="sq", tag="sq")
                    ssq = sp.tile([128, 1], F32, name="ssq", tag="ssq")
                    nc.vector.scalar_tensor_tensor(
                        sq[:], xc[:], 1.0, xc[:], mybir.AluOpType.bypass,
                        mybir.AluOpType.mult, accum_out=ssq[:])
                    std = sp.tile([128, 1], F32, name="std", tag="std")
                    nc.scalar.activation(std[:], ssq[:], AF.Sqrt,
                                         bias=eps[:], scale=1.0 / C)
                    rstd = sp.tile([128, 1], F32, name="rstd", tag="rstd")
                    nc.vector.reciprocal(rstd[:], std[:])
                    hb = ap_.tile([128, C], BF16, name=f"h{tt}", tag=f"h{tt}")
                    nc.vector.tensor_scalar_mul(hb[:], xc[:], rstd[:])
                    out.append(hb)
                return out

            def transpose_128(src_ap, dst_ap, eng):
                ptr = psu.tile([128, 128], BF16, name="tr", tag="u")
                nc.tensor.transpose(ptr[:], src_ap, ident[:])
                if eng == 0:
                    nc.scalar.copy(dst_ap, ptr[:])
                else:
                    nc.vector.tensor_copy(dst_ap, ptr[:])

            def transpose_tiles(tiles, nfree, tag):
                """tiles: list of [128, nfree*128] (token-major) ->
                list of nfree tiles [128, len(tiles)*128] (feature-major)."""
                outs = [ap_.tile([128, len(tiles) * 128], BF16, name=f"{tag}{f}", tag=f"{tag}{f}")
                        for f in range(nfree)]
                e = 0
                for i, t in enumerate(tiles):
                    for f in range(nfree):
                        transpose_128(t[:, f * 128:(f + 1) * 128],
                                      outs[f][:, i * 128:(i + 1) * 128], e % 2)
                        e += 1
                return outs

            for tt in range(NTT):
                nc.sync.dma_start(x[tt][:], d["x0"][tt])
            for li in range(L):
              with nc.named_scope(f"layer{li}"):
                aw = wp.tile([128, NF, 3 * C], BF16, name="aw", tag="aw", bufs=1)
                nc.sync.dma_start(aw[:], d["aw"][li].rearrange(
                    "(f p) n -> p f n", p=128))
                ala = wp.tile([128, NF, R], BF16, name="ala", tag="ala", bufs=2)
                nc.sync.dma_start(ala[:], d["ala"][li].rearrange(
                    "(f p) n -> p f n", p=128))
                alb = wp.tile([R, 3 * C], BF16, name="alb", tag="alb", bufs=2)
                nc.sync.dma_start(alb[:], d["alb"][li])
                pw = wp.tile([128, NF, C], BF16, name="pw", tag="pw", bufs=2)
                nc.sync.dma_start(pw[:], d["pw"][li].rearrange(
                    "(f p) n -> p f n", p=128))
                pla = wp.tile([128, NF, R], BF16, name="pla", tag="pla", bufs=2)
                nc.sync.dma_start(pla[:], d["pla"][li].rearrange(
                    "(f p) n -> p f n", p=128))
                plb = wp.tile([R, C], BF16, name="plb", tag="plb", bufs=2)
                nc.sync.dma_start(plb[:], d["plb"][li])
                fw = wp.tile([128, NF, 4 * C], BF16, name="fw", tag="fw", bufs=2)
                nc.sync.dma_start(fw[:], d["fw"][li].rearrange(
                    "(f p) n -> p f n", p=128))
                mw = wp.tile([128, 16, C], BF16, name="mw", tag="mw", bufs=2)
                nc.sync.dma_start(mw[:], d["mw"][li].rearrange(
                    "(f p) n -> p f n", p=128))

                # ---- attention ----
                h = layernorm(x)
                hT = transpose_tiles(h, NF, "hT")

                # LoRA down-proj: z [R, TO]
                pz = psu.tile([R, TO], F32, name="z", tag="u")
                for f in range(NF):
                    nc.tensor.matmul(pz[:], ala[:, f, :], hT[f][:],
                                     start=(f == 0), stop=(f == NF - 1))
                z = sp.tile([R, TO], BF16, name="z", tag="z")
                nc.scalar.copy(z[:], pz[:])

                # qkvT [3C, TO] feature-major; K and V chunks first so the
                # AllGather can launch while Q is still being computed.
                qT = [ap_.tile([128, TO], BF16, name=f"qT{f}", tag=f"qT{f}") for f in range(NF)]
                vT = [ap_.tile([128, TO], BF16, name=f"vT{f}", tag=f"vT{f}") for f in range(NF)]
                kt_own = ap_.tile([128, NF, TO], BF16, name="kt_own", tag="kt_own")
                for ch in list(range(NF, 3 * NF)) + list(range(NF)):
                    pq = psu.tile([128, TO], F32, name="mm256", tag="u")
                    for f in range(NF):
                        nc.tensor.matmul(pq[:], aw[:, f, ch * 128:(ch + 1) * 128],
                                         hT[f][:], start=(f == 0), stop=False)
                    nc.tensor.matmul(pq[:], alb[:, ch * 128:(ch + 1) * 128],
                                     z[:], start=False, stop=True)
                    if ch < NF:
                        nc.vector.tensor_copy(qT[ch][:], pq[:])
                    elif ch < 2 * NF:
                        nc.scalar.copy(kt_own[:, ch - NF, :], pq[:])
                    else:
                        nc.vector.tensor_copy(vT[ch - 2 * NF][:], pq[:])

                # K AllGather launches as soon as kt_own is ready; the V
                # gather (which additionally waits on the v transposes)
                # overlaps the score matmuls that only need K.
                cinK = dp.tile([128, NF, TO], BF16, name="cinK", tag="cinK")
                coutK = dp.tile([4, 128, NF, TO], BF16, name="coutK",
                                tag="coutK")
                nc.sync.dma_start(cinK[:], kt_own[:])
                nc.gpsimd.collective_compute(
                    "AllGather", mybir.AluOpType.bypass,
                    ins=[cinK.opt()], outs=[coutK.opt()],
                    replica_groups=[[0, 1, 2, 3], [4, 5, 6, 7]],
                )

                # V own -> token-major [TO, C] stored flat [128, (t c)]
                v_own = ap_.tile([128, NTT * C], BF16, name="v_own",
                                 tag="v_own")
                e = 0
                for f in range(NF):
                    for t in range(NTT):
                        transpose_128(vT[f][:, t * 128:(t + 1) * 128],
                                      v_own[:, t * C + f * 128:
                                            t * C + (f + 1) * 128], e % 2)
                        e += 1
                cinV = dp.tile([128, NF, TO], BF16, name="cinV", tag="cinV")
                coutV = dp.tile([4, 128, NF, TO], BF16, name="coutV",
                                tag="coutV")
                nc.sync.dma_start(
                    cinV[:], v_own[:].rearrange("p (f q) -> p f q", f=NF))
                nc.gpsimd.collective_compute(
                    "AllGather", mybir.AluOpType.bypass,
                    ins=[cinV.opt()], outs=[coutV.opt()],
                    replica_groups=[[0, 1, 2, 3], [4, 5, 6, 7]],
                )
                for r in range(4):
                    nc.sync.dma_start(
                        kt_all[:, :, r * TO:(r + 1) * TO], coutK[r])
                for r in range(4):
                    nc.sync.dma_start(
                        v_aug[:, 2 * r:2 * r + 2, :, 0:HD],
                        coutV[r].rearrange(
                            "p (t a) (x e) -> p t (a x) e", t=NTT, x=NF))

                # attention: ST [k, q] per head per k-chunk; P=exp*mask; AV
                y_sb = [ap_.tile([128, C], BF16, name=f"y{tt}", tag=f"y{tt}")
                        for tt in range(NTT)]
                pyav = [[None] * 2 for _ in range(NTT)]
                for hh in range(H):
                    f, po = hh // 2, (hh % 2) * HD
                    pts = []
                    for kp in range(T // 256):
                        pst = psu.tile([128, 2, TO], F32, name="mm256",
                                       tag="u")
                        for j in range(2):
                            kc = 2 * kp + j
                            nc.tensor.matmul(
                                pst[:, j, :],
                                kt_all[po:po + HD, f,
                                       kc * 128:(kc + 1) * 128],
                                qT[f][po:po + HD, :],
                                start=True, stop=True)
                        pt = ap3.tile([128, 2, TO], BF16, name="pt",
                                      tag="pt", bufs=14)
                        nc.scalar.activation(pt[:], pst[:], AF.Exp,
                                             bias=zt[:])
                        nc.vector.tensor_mul(pt[:], pt[:],
                                             maskT[:, 2 * kp:2 * kp + 2, :])
                        pts.append(pt)
                    for tt in range(NTT):
                        hb, hi = hh // 4, hh % 4
                        if hi == 0:
                            pyav[tt][hb] = psu.tile([128, 4, HD + 1], F32,
                                                      name="yav", tag="u")
                        for kc in range(T // 128):
                            nc.tensor.matmul(
                                pyav[tt][hb][:, hi, :],
                                pts[kc // 2][:, kc % 2,
                                             tt * 128:(tt + 1) * 128],
                                v_aug[:, kc, hh, :],
                                start=(kc == 0), stop=(kc == T // 128 - 1))
                        if hi == 3:
                            sums = sp.tile([128, 4], F32, name="sums", tag="sums")
                            for j in range(4):
                                nc.scalar.copy(sums[:, j:j + 1],
                                               pyav[tt][hb][:, j, HD:HD + 1])
                            rec = sp.tile([128, 4], F32, name="rec", tag="rec")
                            nc.vector.reciprocal(rec[:], sums[:])
                            for j in range(4):
                                hj = hb * 4 + j
                                nc.vector.tensor_scalar_mul(
                                    y_sb[tt][:, hj * HD:(hj + 1) * HD],
                                    pyav[tt][hb][:, j, 0:HD],
                                    rec[:, j:j + 1])

                # proj + LoRA + residual
                yT = transpose_tiles(y_sb, NF, "yT")
                pz2 = psu.tile([R, TO], F32, name="z", tag="u")
                for f in range(NF):
                    nc.tensor.matmul(pz2[:], pla[:, f, :], yT[f][:],
                                     start=(f == 0), stop=(f == NF - 1))
                z2 = sp.tile([R, TO], BF16, name="z", tag="z")
                nc.scalar.copy(z2[:], pz2[:])
                for tt in range(NTT):
                    pp_ = psu.tile([128, C], F32, name="mm512", tag="u")
                    for f in range(NF):
                        nc.tensor.matmul(pp_[:], yT[f][:, tt * 128:(tt + 1) * 128],
                                         pw[:, f, :], start=(f == 0), stop=False)
                    nc.tensor.matmul(pp_[:], z2[:, tt * 128:(tt + 1) * 128],
                                     plb[:], start=False, stop=True)
                    nc.vector.tensor_add(x[tt][:], x[tt][:], pp_[:])

                # ---- MLP ----
                h2 = layernorm(x)
                h2T = transpose_tiles(h2, NF, "h2T")
                # fc output computed feature-major: mF [128, 16, TO],
                # g-th 128-slice of the 2048 hidden dim; no transposes.
                mF = ap_.tile([128, 16, TO], BF16, name="mF", tag="mF")
                for gp in range(8):
                    pf = psu.tile([128, 2, TO], F32, name="mm512", tag="u")
                    for j in range(2):
                        g = 2 * gp + j
                        for f in range(NF):
                            nc.tensor.matmul(
                                pf[:, j, :],
                                fw[:, f, g * 128:(g + 1) * 128],
                                h2T[f][:],
                                start=(f == 0), stop=(f == NF - 1))
                    nc.scalar.activation(mF[:, 2 * gp:2 * gp + 2, :], pf[:],
                                         AF.Gelu_apprx_tanh, bias=zt[:])
                for tt in range(NTT):
                    pm = psu.tile([128, C], F32, name="mm512", tag="u")
                    for f in range(16):
                        nc.tensor.matmul(pm[:], mF[:, f, tt * 128:(tt + 1) * 128],
                                         mw[:, f, :],
                                         start=(f == 0), stop=(f == 15))
                    nc.vector.tensor_add(x[tt][:], x[tt][:], pm[:])
                    if debug:
                        nc.sync.dma_start(xdbg[li, tt], x[tt][:])

            # ---- final LN + all-core AllGather of xT + head ----
            with nc.named_scope("head"):
                xf = layernorm(x)
                xfT = transpose_tiles(xf, NF, "xfT")
                cinF = dp.tile([NF, 128, TO], BF16, name="cinF", tag="cinF")
                coutF = dp.tile([8, NF, 128, TO], BF16, name="coutF", tag="coutF",
                                 addr_space="Shared")
                for f in range(NF):
                    nc.sync.dma_start(cinF[f], xfT[f][:])
                nc.gpsimd.collective_compute(
                    "AllGather", mybir.AluOpType.bypass,
                    ins=[cinF.opt()], outs=[coutF.opt()],
                    replica_groups=[[0, 1, 2, 3, 4, 5, 6, 7]],
                )
                xT_all = pp.tile([128, NF, 8 * TO], BF16, name="xta", tag="xta")
                for r in range(8):
                    nc.sync.dma_start(xT_all[:, :, r * TO:(r + 1) * TO],
                                      coutF[r].rearrange("f p q -> p f q"))
                for ch in range(NVCH):
                    nch = min(512, VC - ch * 512)
                    hwt = wp.tile([128, NF, 512], BF16, name="hw", tag="hw", bufs=3)
                    nc.sync.dma_start(
                        hwt[:, :, 0:nch],
                        d["hw"][:, ch * 512:ch * 512 + nch].rearrange(
                            "(f p) n -> p f n", p=128))
                    for tg in range(4):
                        lo = ap3.tile([128, 4, 512], BF16, name="lo", tag="lo",
                                      bufs=2)
                        for ti in range(4):
                            tt = tg * 4 + ti
                            pl = psu.tile([128, 512], F32, name="mm512", tag="u")
                            for f in range(NF):
                                nc.tensor.matmul(
                                    pl[:, 0:nch],
                                    xT_all[:, f, tt * 128:(tt + 1) * 128],
                                    hwt[:, f, 0:nch],
                                    start=(f == 0), stop=(f == NF - 1))
                            if ti % 2 == 0:
                                nc.scalar.copy(lo[:, ti, 0:nch], pl[:, 0:nch])
                            else:
                                nc.vector.tensor_copy(lo[:, ti, 0:nch], pl[:, 0:nch])
                        nc.sync.dma_start(
                            y_d[tg * 512:(tg + 1) * 512,
                                ch * 512:ch * 512 + nch].rearrange(
                                    "(t p) n -> p t n", p=128),
                            lo[:, :, 0:nch])

    nc.compile()
    return nc


def _bf(a):
    return np.ascontiguousarray(a.astype(ml_dtypes.bfloat16))


def host_shards(inputs, debug=False):
    idx = np.asarray(inputs["idx"])
    wte = np.asarray(inputs["wte"], np.float32)
    wpe = np.asarray(inputs["wpe"], np.float32)
    ln1_g = np.asarray(inputs["ln1_g"], np.float32)
    ln2_g = np.asarray(inputs["ln2_g"], np.float32)
    lnf_g = np.asarray(inputs["lnf_g"], np.float32)
    for nm in ("ln1_b", "ln2_b", "fc_b", "mproj_b", "lnf_b"):
        assert np.abs(np.asarray(inputs[nm])).max() == 0.0, f"{nm} nonzero"
    LS = 32.0 / 8.0
    qs = 1.0 / math.sqrt(HD)

    aw = np.empty((L, C, 3 * C), np.float32)
    ala = np.empty((L, C, R), np.float32)
    alb = np.empty((L, R, 3 * C), np.float32)
    pw = np.empty((L, C, C), np.float32)
    pla = np.empty((L, C, R), np.float32)
    plb = np.empty((L, R, C), np.float32)
    fw = np.empty((L, C, 4 * C), np.float32)
    mw = np.empty((L, 4 * C, C), np.float32)
    for i in range(L):
        a = (np.asarray(inputs["attn_w"][i], np.float32) * ln1_g[i][None, :]).T
        a = a.copy()
        a[:, :C] *= qs
        aw[i] = a
        ala[i] = (np.asarray(inputs["attn_lA"][i], np.float32)
                  * ln1_g[i][None, :]).T
        b = np.asarray(inputs["attn_lB"][i], np.float32).T * LS
        b = b.copy()
        b[:, :C] *= qs
        alb[i] = b
        pw[i] = np.asarray(inputs["proj_w"][i], np.float32).T
        pla[i] = np.asarray(inputs["proj_lA"][i], np.float32).T
        plb[i] = np.asarray(inputs["proj_lB"][i], np.float32).T * LS
        fw[i] = (np.asarray(inputs["fc_w"][i], np.float32)
                 * ln2_g[i][None, :]).T
        mw[i] = np.asarray(inputs["mproj_w"][i], np.float32).T
    hwT = (np.asarray(inputs["head_w"], np.float32) * lnf_g[None, :]).T  # [C,V]

    common = dict(aw=_bf(aw), ala=_bf(ala), alb=_bf(alb), pw=_bf(pw),
                  pla=_bf(pla), plb=_bf(plb), fw=_bf(fw), mw=_bf(mw),
                  ident=_bf(np.eye(128, dtype=np.float32)))

    in_maps = []
    for c in range(NCORES):
        g, r = c // 4, c % 4
        sl = slice(r * TO, (r + 1) * TO)
        x0 = wte[idx[g, sl]] + wpe[sl]
        x0 = np.ascontiguousarray(x0.reshape(NTT, 128, C), np.float32)
        # masks[kc, kk, qq]: 1 where key (kc*128+kk) <= query (r*TO+qq)
        kglob = (np.arange(T).reshape(8, 128))[:, :, None]
        qglob = r * TO + np.arange(TO)[None, None, :]
        masks = np.where(kglob <= qglob, 1.0, 0.0).astype(np.float32)
        hw = np.zeros((C, VC), np.float32)
        lo, hi = c * VC, min((c + 1) * VC, V)
        hw[:, 0:hi - lo] = hwT[:, lo:hi]
        m = dict(common)
        m.update(x0=x0, masks=_bf(masks), hw=_bf(hw))
        in_maps.append(m)
    return in_maps


def kernel(**inputs):
    if "nc" not in _CACHE:
        _CACHE["nc"] = build_nc(debug=False)
    nc = _CACHE["nc"]
    in_maps = host_shards(inputs)
    res = bass_utils.run_bass_kernel_spmd(nc, in_maps,
                                          core_ids=list(range(NCORES)))
    out = np.empty((B * T, V), np.float32)
    for c in range(NCORES):
        lo, hi = c * VC, min((c + 1) * VC, V)
        out[:, lo:hi] = res.results[c]["y"][:, 0:hi - lo].astype(np.float32)
    return out.reshape(B, T, V)
